# revision 17
# baseline (speedup 1.0000x reference)
"""GRU cell kernel for Trainium2, data-parallel across 8 NeuronCores.

Reference computation (per batch row):
    concat = [h_prev, x]                       # [B, 2048]
    z = sigmoid(concat @ W_z.T + b_z)          # [B, 1024]
    r = sigmoid(concat @ W_r.T + b_r)
    h_tilde = tanh([r*h_prev, x] @ W_h.T + b_h)
    h = (1-z)*h_prev + z*h_tilde

Sharding: batch dim (8192) split 1024/core; weights replicated.
Layout on device is feature-major ([feature, batch]) so the batch is the
matmul moving dimension (N=512 per PSUM bank) and the hidden units are the
PSUM partition dim. Host transposes in/out.

mm_dtype selects the matmul precision:
  f32r — TF32 PE mode, fp32 storage (rel err ~1e-4)
  bf16 — bf16 operands (weights/acts), fp32 h_prev kept for elementwise
  f32  — true fp32 matmuls (4x slower on PE)
"""

import numpy as np

import concourse.bacc as bacc
import concourse.bass as bass
import concourse.mybir as mybir
import concourse.tile as tile
from concourse import bass_utils

P = 128
B = 8192
I = 1024
H = 1024
K = I + H            # 2048 contraction
NCORES = 8
BS = B // NCORES     # 1024 batch rows per core
MT = H // P          # 8 m-tiles (hidden units)
KT = K // P          # 16 k-chunks
NFREE = 512          # matmul moving free dim (one PSUM bank of fp32)
NT = BS // NFREE     # 2 n-tiles per core

F32 = mybir.dt.float32
F32R = mybir.dt.float32r
BF16 = mybir.dt.bfloat16

AF = mybir.ActivationFunctionType


def build_kernel(mm_dtype: str = "f32r"):
    """Build the per-core Bass kernel. Returns compiled nc."""
    mdt = {"f32r": F32R, "f32": F32, "bf16": BF16}[mm_dtype]
    bf16 = mm_dtype == "bf16"
    nc = bacc.Bacc("TRN2", target_bir_lowering=False, debug=False)

    # DRAM I/O (per-core shapes). Matmul-feeding tensors carry the matmul
    # dtype (f32r is the same bits as f32 on the host side).
    xT = nc.dram_tensor("xT", [I, BS], mdt, kind="ExternalInput").ap()
    hT = nc.dram_tensor("hT", [H, BS], mdt, kind="ExternalInput").ap()
    if bf16:  # separate fp32 copy of h_prev for the elementwise path
        hTf = nc.dram_tensor("hTf", [H, BS], F32, kind="ExternalInput").ap()
    Wz = nc.dram_tensor("Wz", [MT, P, K], mdt, kind="ExternalInput").ap()
    Wr = nc.dram_tensor("Wr", [MT, P, K], mdt, kind="ExternalInput").ap()
    Wh = nc.dram_tensor("Wh", [MT, P, K], mdt, kind="ExternalInput").ap()
    bz = nc.dram_tensor("bz", [P, MT], F32, kind="ExternalInput").ap()
    br = nc.dram_tensor("br", [P, MT], F32, kind="ExternalInput").ap()
    bh = nc.dram_tensor("bh", [P, MT], F32, kind="ExternalInput").ap()
    out = nc.dram_tensor("out", [H, BS], F32, kind="ExternalOutput").ap()

    def ew(ap):
        """fp32 view of an f32r AP for elementwise use (same bits)."""
        return ap.bitcast(F32) if mdt == F32R else ap

    with tile.TileContext(nc) as tc:
        with (
            tc.tile_pool(name="acts", bufs=1) as acts,
            tc.tile_pool(name="gates", bufs=1) as gates,
            tc.tile_pool(name="wpool", bufs=4) as wpool,
            tc.tile_pool(name="opool", bufs=6) as opool,
            tc.tile_pool(name="ppool", bufs=6, space="PSUM") as ppool,
        ):
            # Biases first — they gate the first sigmoid (psum recycling).
            # Scalar HWDGE ring so they don't sit behind the act loads.
            bz_sb = acts.tile([P, MT], F32)
            br_sb = acts.tile([P, MT], F32)
            bh_sb = acts.tile([P, MT], F32)
            nc.scalar.dma_start(br_sb[:], br)
            nc.scalar.dma_start(bz_sb[:], bz)
            nc.scalar.dma_start(bh_sb[:], bh)

            # First two weight tiles go at the HEAD of the sync HWDGE ring:
            # within a ring DMAs drain FIFO, so they get full SDMA bandwidth
            # before the act loads start, instead of a round-robin share.
            # (The SWDGE queue used for the remaining tiles takes ~6us to
            # spin up anyway.)
            w_first = [wpool.tile([P, K], mdt, tag="w", name=f"wf{i}")
                       for i in range(2)]
            nc.sync.dma_start(w_first[0][:], Wr[0])
            nc.sync.dma_start(w_first[1][:], Wr[1])

            # Pre-warm the ACT sigmoid/tanh table set during the DMA fill so
            # the first real sigmoid doesn't pay the ~2.7us ACT_TABLE_LOAD.
            # Reads its own uninitialized tile — no DMA dependency, result
            # discarded — so it cannot stall the scalar ring's weight DMAs.
            warm = acts.tile([P, 1], F32)
            nc.scalar.activation(warm[:], warm[:], AF.Sigmoid)

            # Persistent activations, feature-major: [p, ko, batch]
            xT_sb = acts.tile([P, I // P, BS], mdt)
            hT_sb = acts.tile([P, H // P, BS], mdt)
            hTf_sb = (acts.tile([P, H // P, BS], F32, name="hTf_sb")
                      if bf16 else None)
            # Load per (k-chunk, batch-half), n=0 halves first, so the first
            # PSUM groups (n=0) are gated on half the act bytes. Weight DMAs
            # ride the idle GpSimd SWDGE queue so they don't serialize with
            # act loads or compute issue.
            xT_r = xT.rearrange("(ko p) b -> p ko b", p=P)
            hT_r = hT.rearrange("(ko p) b -> p ko b", p=P)
            hTf_r = hTf.rearrange("(ko p) b -> p ko b", p=P) if bf16 else None
            for n in range(NT):
                ns = slice(n * NFREE, (n + 1) * NFREE)
                for ko in range(H // P):
                    nc.sync.dma_start(hT_sb[:, ko, ns], hT_r[:, ko, ns])
                for ko in range(I // P):
                    nc.sync.dma_start(xT_sb[:, ko, ns], xT_r[:, ko, ns])
            if bf16:
                for ko in range(H // P):
                    nc.sync.dma_start(hTf_sb[:, ko, :], hTf_r[:, ko, :])

            # Gate results, feature-major
            z_sb = gates.tile([P, MT, BS], F32)
            rh_sb = gates.tile([P, MT, BS], mdt)

            def hprev_ew(mt, ns):
                """fp32-precision h_prev slice for elementwise use."""
                if bf16:
                    return hTf_sb[:, mt, ns]
                return ew(hT_sb[:, mt, ns])

            def rhs_chunk(k, n, stage):
                """Moving operand [128, 512] for contraction chunk k."""
                if k < H // P:
                    src = rh_sb if stage == "h" else hT_sb
                    return src[:, k, n * NFREE:(n + 1) * NFREE]
                return xT_sb[:, k - H // P, n * NFREE:(n + 1) * NFREE]

            def gate(stage, Wd, b_sb):
                for mt in range(MT):
                    if stage == "r" and mt < 2:
                        w_sb = w_first[mt]
                    else:
                        w_sb = wpool.tile([P, K], mdt, tag="w")
                        nc.gpsimd.dma_start(w_sb[:], Wd[mt])
                    for n in range(NT):
                        ps = ppool.tile([P, NFREE], F32, tag="ps")
                        for k in range(KT):
                            nc.tensor.matmul(
                                ps,
                                w_sb[:, k * P:(k + 1) * P],
                                rhs_chunk(k, n, stage),
                                start=(k == 0),
                                stop=(k == KT - 1),
                            )
                        ns = slice(n * NFREE, (n + 1) * NFREE)
                        bias = b_sb[:, mt:mt + 1]
                        if stage == "r":
                            # r -> rh = r * h_prev, written as matmul dtype
                            r_tmp = opool.tile([P, NFREE], F32, tag="rt")
                            nc.scalar.activation(
                                r_tmp, ps, AF.Sigmoid, bias=bias)
                            nc.vector.tensor_mul(
                                rh_sb[:, mt, ns], r_tmp, hprev_ew(mt, ns))
                        elif stage == "z":
                            nc.scalar.activation(
                                z_sb[:, mt, ns], ps, AF.Sigmoid, bias=bias)
                        else:  # h: h = h_prev + z*(tanh(pre) - h_prev)
                            ht = opool.tile([P, NFREE], F32, tag="ht")
                            nc.scalar.activation(ht, ps, AF.Tanh, bias=bias)
                            nc.vector.tensor_sub(ht, ht, hprev_ew(mt, ns))
                            nc.vector.tensor_mul(ht, ht, z_sb[:, mt, ns])
                            nc.vector.tensor_add(ht, ht, hprev_ew(mt, ns))
                            nc.sync.dma_start(
                                out[mt * P:(mt + 1) * P, ns], ht)

            gate("r", Wr, br_sb)
            gate("z", Wz, bz_sb)
            gate("h", Wh, bh_sb)

    nc.compile()
    return nc


def _prep_inputs(x, h_prev, W_z, b_z, W_r, b_r, W_h, b_h, np_dtype=np.float32):
    """Host-side relayout: feature-major activations, m-tiled weights."""
    bf16 = np_dtype != np.float32

    def prep_w(W):
        # want w[mt, p, ko*128+m] = W[mt*128+m, ko*128+p]
        W4 = W.reshape(MT, P, KT, P)          # [mt, m, ko, p]
        return np.ascontiguousarray(
            W4.transpose(0, 3, 2, 1)).reshape(MT, P, K).astype(np_dtype)

    def prep_b(b):
        return np.ascontiguousarray(b.reshape(MT, P).T)

    xT = np.ascontiguousarray(x.T).astype(np_dtype)       # [I, B]
    hTf = np.ascontiguousarray(h_prev.T)                  # [H, B] f32
    hT = hTf.astype(np_dtype)
    shared = {
        "Wz": prep_w(W_z), "Wr": prep_w(W_r), "Wh": prep_w(W_h),
        "bz": prep_b(b_z), "br": prep_b(b_r), "bh": prep_b(b_h),
    }
    in_maps = []
    for c in range(NCORES):
        bs = slice(c * BS, (c + 1) * BS)
        m = dict(shared)
        m["xT"] = np.ascontiguousarray(xT[:, bs])
        m["hT"] = np.ascontiguousarray(hT[:, bs])
        if bf16:
            m["hTf"] = np.ascontiguousarray(hTf[:, bs])
        in_maps.append(m)
    return in_maps


def run(inputs, mm_dtype="f32r", trace=False, **run_kwargs):
    """Compile + run on 8 cores. Returns (output [B,H] f32, BassKernelResults)."""
    import ml_dtypes
    nc = build_kernel(mm_dtype)
    np_dtype = ml_dtypes.bfloat16 if mm_dtype == "bf16" else np.float32
    in_maps = _prep_inputs(**inputs, np_dtype=np_dtype)
    res = bass_utils.run_bass_kernel_spmd(
        nc, in_maps, core_ids=list(range(NCORES)), trace=trace, **run_kwargs)
    outT = np.concatenate(
        [res.results[c]["out"] for c in range(NCORES)], axis=1)  # [H, B]
    return np.ascontiguousarray(outT.T), res


def kernel(**inputs) -> np.ndarray:
    out, _ = run(inputs)
    return out
